# revision 1
# baseline (speedup 1.0000x reference)
"""GroupedQueryAttention TRN2 kernel — 8-core SPMD (batch x tensor-parallel).

Sharding: core c = 2*b + tp. Each core handles batch b and kv-heads
{2tp, 2tp+1} (with both query groups per kv head co-located). Host folds
mproj into Wk, vproj into Wv, the 1/sqrt(dq) scale into Wq, and biases
either into per-partition on-chip adds (q, k) or a host-side output
constant (v, o). Each core returns a partial y.T [512, T]; host sums the
two tp partials per batch and transposes.

On-chip dataflow (per core, all matmuls bf16, fp32 PSUM):
  qT[p]  [64, T]  = Wq_p^T-stationary matmuls over Q^T chunks (p = 2*h+g)
  kT[h]  [64, T]  = folded-Wk matmuls over K^T chunks
  v_sb   [128,130] per n-tile: [v_h0 | ones | v_h1 | ones] (aug column
                   yields the softmax denominator during AV)
  scores S^T [n=128, t<=512] = kT-chunk-stationary x qT-moving; exp on ACT
  (no max subtraction: folded scores are O(0.05), exp is safe); causal
  masking = block-triangular loop bounds + gpsimd affine_select on the
  128-wide diagonal subtiles.
  AV: OT[65, t] += V_aug-stationary x P^T-moving (row 64 = denominator)
  normalize: reciprocal + K=1 broadcast-matmul (ones row at partition 64)
  + DVE multiply; g=1 tiles shifted to partitions 64-127 via SBUF DMA.
  Wo: y^T [o=128, t] = Wo^T-chunk-stationary x OT-moving.
"""

import numpy as np
import ml_dtypes

import concourse.bass as bass
import concourse.bacc as bacc
import concourse.mybir as mybir
from concourse import tile
from concourse.bass_utils import run_bass_kernel_spmd

B, T, D = 4, 2048, 512
HQ, HKV = 8, 4
DQ, DKV = 64, 128
G = 2
NCORES = 8
BF16NP = ml_dtypes.bfloat16

f32 = mybir.dt.float32
bf16 = mybir.dt.bfloat16
EXP = mybir.ActivationFunctionType.Exp
GE = mybir.AluOpType.is_ge


def build_module(t=T, debug_outs=False):
    assert t % 512 == 0
    tb_n = t // 512   # 512-wide t blocks
    nt_n = t // 128   # 128-wide n tiles

    nc = bacc.Bacc("TRN2", target_bir_lowering=False, debug=False)

    qt_d = nc.dram_tensor("qt", [512, t], bf16, kind="ExternalInput").ap()
    kt_d = nc.dram_tensor("kt", [512, t], bf16, kind="ExternalInput").ap()
    vt_d = nc.dram_tensor("vt", [512, t], bf16, kind="ExternalInput").ap()
    wq_d = nc.dram_tensor("wq", [512, 256], bf16, kind="ExternalInput").ap()
    wk_d = nc.dram_tensor("wk", [512, 128], bf16, kind="ExternalInput").ap()
    wv_d = nc.dram_tensor("wv", [512, 130], bf16, kind="ExternalInput").ap()
    wo_d = nc.dram_tensor("wo", [256, 512], bf16, kind="ExternalInput").ap()
    bq_d = nc.dram_tensor("bq", [64, 4], f32, kind="ExternalInput").ap()
    bk_d = nc.dram_tensor("bk", [64, 2], f32, kind="ExternalInput").ap()
    yt_d = nc.dram_tensor("yt", [512, t], bf16, kind="ExternalOutput").ap()
    if debug_outs:
        dbg = {k: nc.dram_tensor(k, sh, bf16, kind="ExternalOutput").ap()
               for k, sh in [("dq0", [64, t]), ("dk0", [64, t]),
                             ("dv", [128, (t // 128) * 130]),
                             ("do0", [128, t]), ("do1", [128, t]),
                             ("dp", [128, 1024])]}

    with tile.TileContext(nc) as tc:
        with tc.tile_pool(name="const", bufs=1) as cpool, \
             tc.tile_pool(name="big", bufs=1) as bigp:
            wq_sb = cpool.tile([128, 4 * 256], bf16, tag="wq", name="wq")
            wk_sb = cpool.tile([128, 4 * 128], bf16, tag="wk", name="wk")
            wv_sb = cpool.tile([128, 4 * 130], bf16, tag="wv", name="wv")
            wo_sb = cpool.tile([128, 2 * 512], bf16, tag="wo", name="wo")
            bq_sb = cpool.tile([64, 4], f32, tag="bq", name="bq")
            bk_sb = cpool.tile([64, 2], f32, tag="bk", name="bk")
            ones_sb = cpool.tile([65, 128], bf16, tag="ones", name="ones")
            for c in range(4):
                nc.sync.dma_start(wq_sb[:, c * 256:(c + 1) * 256],
                                  wq_d[c * 128:(c + 1) * 128, :])
                nc.sync.dma_start(wk_sb[:, c * 128:(c + 1) * 128],
                                  wk_d[c * 128:(c + 1) * 128, :])
                nc.sync.dma_start(wv_sb[:, c * 130:(c + 1) * 130],
                                  wv_d[c * 128:(c + 1) * 128, :])
            for h in range(2):
                nc.sync.dma_start(wo_sb[:, h * 512:(h + 1) * 512],
                                  wo_d[h * 128:(h + 1) * 128, :])
            nc.sync.dma_start(bq_sb[:, :], bq_d[:, :])
            nc.sync.dma_start(bk_sb[:, :], bk_d[:, :])
            nc.vector.memset(ones_sb[64:65, :], 1.0)

            qt_sb = bigp.tile([128, 4 * t], bf16, tag="qt", name="qt")
            kt_sb = bigp.tile([128, 4 * t], bf16, tag="kt", name="kt")
            vt_sb = bigp.tile([128, 4 * t], bf16, tag="vt", name="vt")
            for c in range(4):
                nc.sync.dma_start(qt_sb[:, c * t:(c + 1) * t],
                                  qt_d[c * 128:(c + 1) * 128, :])
                nc.sync.dma_start(kt_sb[:, c * t:(c + 1) * t],
                                  kt_d[c * 128:(c + 1) * 128, :])
                nc.sync.dma_start(vt_sb[:, c * t:(c + 1) * t],
                                  vt_d[c * 128:(c + 1) * 128, :])

            qT = [bigp.tile([64, t], bf16, tag=f"qT{p}", name=f"qT{p}") for p in range(4)]
            kT = [bigp.tile([64, t], bf16, tag=f"kT{h}", name=f"kT{h}") for h in range(2)]
            v_sb = bigp.tile([128, nt_n * 130], bf16, tag="v", name="v")
            oT = [bigp.tile([128, t], bf16, tag=f"oT{h}", name=f"oT{h}") for h in range(2)]

            # ---- phase 1: projections ----
            with tc.tile_pool(name="ps1", bufs=3, space="PSUM") as ps1:
                for p in range(4):
                    for tb in range(tb_n):
                        ps = ps1.tile([64, 512], f32, tag="ps", name="ps")
                        for c in range(4):
                            nc.tensor.matmul(
                                ps[:, :],
                                wq_sb[:, c * 256 + p * 64: c * 256 + (p + 1) * 64],
                                qt_sb[:, c * t + tb * 512: c * t + (tb + 1) * 512],
                                start=(c == 0), stop=(c == 3))
                        nc.vector.tensor_scalar_add(
                            qT[p][:, tb * 512:(tb + 1) * 512], ps[:, :],
                            bq_sb[:, p:p + 1])
                for h in range(2):
                    for tb in range(tb_n):
                        ps = ps1.tile([64, 512], f32, tag="ps", name="ps")
                        for c in range(4):
                            nc.tensor.matmul(
                                ps[:, :],
                                wk_sb[:, c * 128 + h * 64: c * 128 + (h + 1) * 64],
                                kt_sb[:, c * t + tb * 512: c * t + (tb + 1) * 512],
                                start=(c == 0), stop=(c == 3))
                        nc.vector.tensor_scalar_add(
                            kT[h][:, tb * 512:(tb + 1) * 512], ps[:, :],
                            bk_sb[:, h:h + 1])
                for nt in range(nt_n):
                    ps = ps1.tile([128, 130], f32, tag="psv", name="psv")
                    for c in range(4):
                        nc.tensor.matmul(
                            ps[:, :],
                            vt_sb[:, c * t + nt * 128: c * t + (nt + 1) * 128],
                            wv_sb[:, c * 130:(c + 1) * 130],
                            start=(c == 0), stop=(c == 3))
                    nc.vector.tensor_copy(v_sb[:, nt * 130:(nt + 1) * 130], ps[:, :])
                    nc.gpsimd.memset(v_sb[:, nt * 130 + 64: nt * 130 + 65], 1.0)
                    nc.gpsimd.memset(v_sb[:, nt * 130 + 129: nt * 130 + 130], 1.0)

            # ---- phase 2: attention ----
            with tc.tile_pool(name="s2", bufs=2, space="PSUM") as s2p, \
                 tc.tile_pool(name="otp", bufs=1, space="PSUM") as otp, \
                 tc.tile_pool(name="bcp", bufs=1, space="PSUM") as bcp, \
                 tc.tile_pool(name="ptp", bufs=4) as ptp, \
                 tc.tile_pool(name="npool", bufs=4) as npl:
                for h in range(2):
                    for tb in range(tb_n):
                        otg = [otp.tile([65, 512], f32, tag=f"ot{g}", name=f"ot{g}")
                               for g in range(2)]
                        nch = 4 * (tb + 1)
                        for i in range(nch):
                            n0 = 128 * i
                            lo = max(0, n0 - tb * 512)
                            s2 = s2p.tile([128, 1024], f32, tag="s2", name="s2")
                            for g in range(2):
                                nc.tensor.matmul(
                                    s2[:, g * 512 + lo:(g + 1) * 512],
                                    kT[h][:, n0:n0 + 128],
                                    qT[2 * h + g][:, tb * 512 + lo:(tb + 1) * 512],
                                    start=True, stop=True)
                            pt = ptp.tile([128, 1024], bf16, tag="pt", name="pt")
                            s2v = s2[:, :].rearrange("p (g m) -> p g m", g=2)[:, :, lo:512]
                            ptv = pt[:, :].rearrange("p (g m) -> p g m", g=2)[:, :, lo:512]
                            nc.scalar.activation(ptv, s2v, EXP)
                            if n0 >= tb * 512:
                                for g in range(2):
                                    sl = pt[:, g * 512 + lo: g * 512 + lo + 128]
                                    nc.gpsimd.affine_select(
                                        out=sl, in_=sl, compare_op=GE, fill=0.0,
                                        base=0, pattern=[[1, 128]],
                                        channel_multiplier=-1)
                            if debug_outs and h == 0 and tb == 0 and i == 0:
                                nc.sync.dma_start(dbg["dp"][:, :], pt[:, :])
                            for g in range(2):
                                nc.tensor.matmul(
                                    otg[g][:, lo:512],
                                    v_sb[:, i * 130 + h * 65: i * 130 + h * 65 + 65],
                                    pt[:, g * 512 + lo:(g + 1) * 512],
                                    start=(i == 0), stop=(i == nch - 1),
                                    skip_group_check=True)
                        for g in range(2):
                            den = npl.tile([65, 1024], f32, tag="den", name="den")
                            nc.vector.tensor_copy(den[64:65, 0:512],
                                                  otg[g][64:65, :])
                            nc.vector.reciprocal(
                                den[64:65, 512:1024], den[64:65, 0:512])
                            denb = npl.tile([65, 512], bf16, tag="denb", name="denb")
                            nc.vector.tensor_copy(denb[64:65, :],
                                                  den[64:65, 512:1024])
                            bc = bcp.tile([128, 512], f32, tag="bc", name="bc")
                            nc.tensor.matmul(bc[:, :], ones_sb[64:65, :],
                                             denb[64:65, :], start=True, stop=True)
                            num = npl.tile([64, 512], bf16, tag="num", name="num")
                            nc.vector.tensor_copy(num[:, :], otg[g][0:64, :])
                            if g == 0:
                                nc.vector.tensor_mul(
                                    oT[h][0:64, tb * 512:(tb + 1) * 512],
                                    num[:, :], bc[0:64, :])
                            else:
                                nm = npl.tile([64, 512], bf16, tag="nm", name="nm")
                                nc.vector.tensor_mul(nm[:, :], num[:, :],
                                                     bc[0:64, :])
                                nc.sync.dma_start(
                                    oT[h][64:128, tb * 512:(tb + 1) * 512],
                                    nm[:, :])

            if debug_outs:
                nc.sync.dma_start(dbg["dq0"][:, :], qT[0][:, :])
                nc.sync.dma_start(dbg["dk0"][:, :], kT[0][:, :])
                nc.sync.dma_start(dbg["dv"][:, :], v_sb[:, :])
                nc.sync.dma_start(dbg["do0"][:, :], oT[0][:, :])
                nc.sync.dma_start(dbg["do1"][:, :], oT[1][:, :])

            # ---- phase 3: output projection ----
            with tc.tile_pool(name="ps3", bufs=4, space="PSUM") as ps3, \
                 tc.tile_pool(name="ys", bufs=4) as ysp:
                for oc in range(4):
                    for tb in range(tb_n):
                        yp = ps3.tile([128, 512], f32, tag="yp", name="yp")
                        for hh in range(2):
                            nc.tensor.matmul(
                                yp[:, :],
                                wo_sb[:, hh * 512 + oc * 128: hh * 512 + (oc + 1) * 128],
                                oT[hh][:, tb * 512:(tb + 1) * 512],
                                start=(hh == 0), stop=(hh == 1))
                        ys = ysp.tile([128, 512], bf16, tag="ys", name="ys")
                        nc.vector.tensor_copy(ys[:, :], yp[:, :])
                        nc.sync.dma_start(
                            yt_d[oc * 128:(oc + 1) * 128, tb * 512:(tb + 1) * 512],
                            ys[:, :])

    nc.compile()
    return nc


def prep_inputs(inputs, t=T):
    """Host-side fold + shard. Returns (in_maps[8], out_const[512] f32)."""
    Q = np.asarray(inputs["Q"], np.float32)
    K = np.asarray(inputs["K"], np.float32)
    V = np.asarray(inputs["V"], np.float32)
    Wq_w = np.asarray(inputs["Wq_w"], np.float32)
    Wq_b = np.asarray(inputs["Wq_b"], np.float32)
    Wk_w = np.asarray(inputs["Wk_w"], np.float32)
    Wk_b = np.asarray(inputs["Wk_b"], np.float32)
    Wv_w = np.asarray(inputs["Wv_w"], np.float32)
    Wv_b = np.asarray(inputs["Wv_b"], np.float32)
    Wo_w = np.asarray(inputs["Wo_w"], np.float32)
    Wo_b = np.asarray(inputs["Wo_b"], np.float32)
    vproj_w = np.asarray(inputs["vproj_w"], np.float32)
    vproj_b = np.asarray(inputs["vproj_b"], np.float32)
    mproj_w = np.asarray(inputs["mproj_w"], np.float32)
    mproj_b = np.asarray(inputs["mproj_b"], np.float32)

    b_n = Q.shape[0]
    s = 1.0 / np.sqrt(np.float32(DQ))

    qt = [np.ascontiguousarray(Q[b, :t].T).astype(BF16NP) for b in range(b_n)]
    kt = [np.ascontiguousarray(K[b, :t].T).astype(BF16NP) for b in range(b_n)]
    vt = [np.ascontiguousarray(V[b, :t].T).astype(BF16NP) for b in range(b_n)]

    per_tp = []
    for tp in range(2):
        wq = np.zeros((512, 256), np.float32)
        bq = np.zeros((64, 4), np.float32)
        wk = np.zeros((512, 128), np.float32)
        bk = np.zeros((64, 2), np.float32)
        wv = np.zeros((512, 130), np.float32)
        wo = np.zeros((256, 512), np.float32)
        for h in range(2):
            hg = 2 * tp + h
            wk_eff = mproj_w @ Wk_w[hg * 128:(hg + 1) * 128, :]
            bk_eff = mproj_w @ Wk_b[hg * 128:(hg + 1) * 128] + mproj_b
            wk[:, h * 64:(h + 1) * 64] = wk_eff.T
            bk[:, h] = bk_eff
            wv_eff = vproj_w @ Wv_w[hg * 128:(hg + 1) * 128, :]
            wv[:, h * 65:h * 65 + 64] = wv_eff.T
            for g in range(2):
                p = 2 * h + g
                hq = g * HKV + hg
                wq[:, p * 64:(p + 1) * 64] = (Wq_w[hq * 64:(hq + 1) * 64, :] * s).T
                bq[:, p] = Wq_b[hq * 64:(hq + 1) * 64] * s
                wo[p * 64:(p + 1) * 64, :] = Wo_w[:, hq * 64:(hq + 1) * 64].T
        per_tp.append(dict(
            wq=wq.astype(BF16NP), wk=wk.astype(BF16NP), wv=wv.astype(BF16NP),
            wo=wo.astype(BF16NP), bq=bq, bk=bk))

    out_const = Wo_b.copy()
    for hq in range(HQ):
        hg = hq % HKV
        bv_eff = vproj_w @ Wv_b[hg * 128:(hg + 1) * 128] + vproj_b
        out_const += Wo_w[:, hq * 64:(hq + 1) * 64] @ bv_eff

    in_maps = []
    for b in range(b_n):
        for tp in range(2):
            w = per_tp[tp]
            in_maps.append(dict(
                qt=qt[b], kt=kt[b], vt=vt[b],
                wq=w["wq"], wk=w["wk"], wv=w["wv"], wo=w["wo"],
                bq=w["bq"], bk=w["bk"]))
    return in_maps, out_const


_NC_CACHE = {}


def get_module(t=T, debug_outs=False):
    key = (t, debug_outs)
    if key not in _NC_CACHE:
        _NC_CACHE[key] = build_module(t, debug_outs)
    return _NC_CACHE[key]


def run_on_cores(inputs, t=T, debug_outs=False, **run_kwargs):
    nc = get_module(t, debug_outs)
    in_maps, out_const = prep_inputs(inputs, t)
    res = run_bass_kernel_spmd(nc, in_maps, core_ids=list(range(NCORES)),
                               **run_kwargs)
    b_n = len(in_maps) // 2
    out = np.empty((b_n, t, D), np.float32)
    for b in range(b_n):
        acc = (res.results[2 * b]["yt"].astype(np.float32)
               + res.results[2 * b + 1]["yt"].astype(np.float32))
        out[b] = acc.T + out_const[None, :]
    return out, res


def kernel(**inputs):
    out, _ = run_on_cores(inputs, t=T)
    return out

